# revision 4
# baseline (speedup 1.0000x reference)
"""Trainium2 Bass kernel for: 3-layer MLP (ReLU after every layer) over
262144x512 rows + global row-sum-mean threshold + mask > threshold.

Sharding: data-parallel over 8 NeuronCores. Rows split 32768/core; the
mask's leading dim split 64/core. MLP weights replicated. One AllReduce
of the per-core row-sum partials produces the global threshold before
the mask comparison.

Self-contained: hardcodes all shapes; no sibling imports.
"""

import numpy as np
import ml_dtypes

import concourse.bacc as bacc
import concourse.mybir as mybir
import concourse.tile as tile
from concourse import bass_utils

CORES = 8
ROWS = 262144
RPC = ROWS // CORES          # 32768 rows per core
D = 512                      # input/output feature dim
H = 1024                     # hidden dim
NB = 512                     # rows per block (matmul moving free dim)
NBLK = RPC // NB             # 64 blocks per core
MF = (512 * 512 * 512) // CORES // 128   # mask free dim per core = 131072
MT = 4096                    # mask tile free dim
NMT = MF // MT               # 32 mask tiles per core

bf16 = mybir.dt.bfloat16
f32 = mybir.dt.float32
u8 = mybir.dt.uint8
Relu = mybir.ActivationFunctionType.Relu
Copy = mybir.ActivationFunctionType.Copy

_NC = None            # cached compiled kernel
LAST_RESULTS = None   # BassKernelResults of the last run (for test harness)
LAST_IN_MAPS = None   # per-core input maps of the last run (for test harness)


def _build():
    nc = bacc.Bacc("TRN2", target_bir_lowering=False, debug=False,
                   num_devices=CORES)

    x = nc.dram_tensor("x", [RPC, D], bf16, kind="ExternalInput").ap()
    w1t = nc.dram_tensor("w1t", [D, H], bf16, kind="ExternalInput").ap()
    w2t = nc.dram_tensor("w2t", [H, H], bf16, kind="ExternalInput").ap()
    w3t = nc.dram_tensor("w3t", [H, D], bf16, kind="ExternalInput").ap()
    b1r = nc.dram_tensor("b1r", [128, H // 128], f32, kind="ExternalInput").ap()
    b2r = nc.dram_tensor("b2r", [128, H // 128], f32, kind="ExternalInput").ap()
    b3r = nc.dram_tensor("b3r", [1, D], bf16, kind="ExternalInput").ap()
    maskd = nc.dram_tensor("maskd", [128, MF], f32, kind="ExternalInput").ap()
    outd = nc.dram_tensor("outd", [RPC, D], f32, kind="ExternalOutput").ap()
    hid = nc.dram_tensor("hid", [128, MF], u8, kind="ExternalOutput").ap()

    with tile.TileContext(nc) as tc:
        with (
            tc.tile_pool(name="const", bufs=1) as const,
            tc.tile_pool(name="wpool", bufs=1) as wpool,
            tc.tile_pool(name="xTp", bufs=8) as xTp,
            tc.tile_pool(name="h1p", bufs=16) as h1p,
            tc.tile_pool(name="h2p", bufs=16) as h2p,
            tc.tile_pool(name="outp", bufs=4) as outp,
            tc.tile_pool(name="misc", bufs=1) as misc,
            tc.tile_pool(name="psA", bufs=3, space="PSUM") as psA,
            tc.tile_pool(name="psB", bufs=3, space="PSUM") as psB,
            tc.tile_pool(name="maskp", bufs=4) as maskp,
            tc.tile_pool(name="hip", bufs=4) as hip,
            tc.tile_pool(name="dram", bufs=1, space="DRAM") as dram,
        ):
            # ---- constants / weights resident in SBUF ----
            ones_inv = const.tile([128, 128], f32)
            nc.any.memset(ones_inv[:], 1.0 / ROWS)
            ones1 = const.tile([1, 128], bf16)
            nc.any.memset(ones1[:], 1.0)
            zerob = const.tile([128, 1], f32)
            nc.any.memset(zerob[:], 0.0)

            b1s = const.tile([128, H // 128], f32)
            nc.sync.dma_start(out=b1s[:], in_=b1r[:, :])
            b2s = const.tile([128, H // 128], f32)
            nc.sync.dma_start(out=b2s[:], in_=b2r[:, :])
            b3s = const.tile([1, D], bf16)
            nc.sync.dma_start(out=b3s[:], in_=b3r[:, :])

            w1s = []
            for k in range(D // 128):
                t = wpool.tile([128, H], bf16, tag=f"w1_{k}")
                nc.sync.dma_start(out=t[:], in_=w1t[k * 128:(k + 1) * 128, :])
                w1s.append(t)
            w2s = []
            for k in range(H // 128):
                t = wpool.tile([128, H], bf16, tag=f"w2_{k}")
                nc.sync.dma_start(out=t[:], in_=w2t[k * 128:(k + 1) * 128, :])
                w2s.append(t)
            w3s = []
            for k in range(H // 128):
                t = wpool.tile([128, D], bf16, tag=f"w3_{k}")
                nc.sync.dma_start(out=t[:], in_=w3t[k * 128:(k + 1) * 128, :])
                w3s.append(t)

            # per-(block,row-tile) row sums, reduced at the end
            rsums = misc.tile([128, NBLK * (NB // 128)], f32)

            # ---- phase 1: MLP over row blocks ----
            for b in range(NBLK):
                r0 = b * NB
                xT = []
                for k in range(D // 128):
                    t = xTp.tile([128, NB], bf16, tag="xT")
                    nc.sync.dma_start(
                        out=t[:], in_=x[r0:r0 + NB, k * 128:(k + 1) * 128],
                        transpose=True)
                    xT.append(t)

                h1 = []
                for m in range(H // 128):
                    ps = psA.tile([128, NB], f32, tag="psA")
                    for k in range(D // 128):
                        nc.tensor.matmul(
                            ps[:], lhsT=w1s[k][:, m * 128:(m + 1) * 128],
                            rhs=xT[k][:], start=(k == 0), stop=(k == D // 128 - 1))
                    t = h1p.tile([128, NB], bf16, tag="h1")
                    nc.scalar.activation(t[:], ps[:], Relu, bias=b1s[:, m:m + 1])
                    h1.append(t)

                h2 = []
                for m in range(H // 128):
                    ps = psA.tile([128, NB], f32, tag="psA")
                    for k in range(H // 128):
                        nc.tensor.matmul(
                            ps[:], lhsT=w2s[k][:, m * 128:(m + 1) * 128],
                            rhs=h1[k][:], start=(k == 0), stop=(k == H // 128 - 1))
                    t = h2p.tile([128, NB], bf16, tag="h2")
                    nc.scalar.activation(t[:], ps[:], Relu, bias=b2s[:, m:m + 1])
                    h2.append(t)

                for r in range(NB // 128):
                    ps = psB.tile([128, D], f32, tag="psB")
                    # bias row via K=1 ones matmul: psum = ones.T @ b3
                    nc.tensor.matmul(ps[:], lhsT=ones1[0:1, :], rhs=b3s[0:1, :],
                                     start=True, stop=False)
                    for k in range(H // 128):
                        nc.tensor.matmul(
                            ps[:], lhsT=h2[k][:, r * 128:(r + 1) * 128],
                            rhs=w3s[k][:], start=False, stop=(k == H // 128 - 1))
                    o = outp.tile([128, D], f32, tag="o")
                    nc.scalar.activation(
                        o[:], ps[:], Relu, bias=zerob[:],
                        accum_out=rsums[:, b * (NB // 128) + r:
                                        b * (NB // 128) + r + 1])
                    rr = r0 + r * 128
                    nc.sync.dma_start(out=outd[rr:rr + 128, :], in_=o[:])

            # ---- threshold: partial -> AllReduce -> broadcast scalar ----
            part = misc.tile([128, 1], f32)
            nc.vector.reduce_sum(part[:], rsums[:], axis=mybir.AxisListType.X)

            bounce_in = dram.tile([128, 1], f32)
            bounce_out = dram.tile([128, 1], f32)
            nc.sync.dma_start(out=bounce_in[:], in_=part[:])
            nc.gpsimd.collective_compute(
                "AllReduce", mybir.AluOpType.add,
                replica_groups=[list(range(CORES))],
                ins=[bounce_in.opt()], outs=[bounce_out.opt()])
            g = misc.tile([128, 1], f32)
            nc.sync.dma_start(out=g[:], in_=bounce_out[:])

            # threshold (= global mean) broadcast to all partitions via
            # all-(1/N) matmul
            ps = psB.tile([128, D], f32, tag="psB")
            nc.tensor.matmul(ps[:, 0:1], lhsT=ones_inv[:], rhs=g[:],
                             start=True, stop=True)
            thr = misc.tile([128, 1], f32)
            nc.scalar.activation(thr[:], ps[:, 0:1], Copy)

            # ---- phase 2: hi_mask = mask > threshold ----
            for t in range(NMT):
                mt = maskp.tile([128, MT], f32, tag="m")
                nc.sync.dma_start(out=mt[:], in_=maskd[:, t * MT:(t + 1) * MT])
                ht = hip.tile([128, MT], u8, tag="hi")
                nc.vector.tensor_scalar(out=ht[:], in0=mt[:], scalar1=thr[:, 0:1],
                                        scalar2=None, op0=mybir.AluOpType.is_gt)
                nc.sync.dma_start(out=hid[:, t * MT:(t + 1) * MT], in_=ht[:])

    nc.compile()
    return nc


def kernel(input, mask, W1, b1, W2, b2, W3, b3):
    global _NC, LAST_RESULTS, LAST_IN_MAPS
    if _NC is None:
        _NC = _build()
    nc = _NC

    x_bf = np.asarray(input, dtype=np.float32).astype(ml_dtypes.bfloat16)
    w1t = np.ascontiguousarray(np.asarray(W1, dtype=np.float32).T).astype(ml_dtypes.bfloat16)
    w2t = np.ascontiguousarray(np.asarray(W2, dtype=np.float32).T).astype(ml_dtypes.bfloat16)
    w3t = np.ascontiguousarray(np.asarray(W3, dtype=np.float32).T).astype(ml_dtypes.bfloat16)
    b1r = np.ascontiguousarray(np.asarray(b1, dtype=np.float32).reshape(H // 128, 128).T)
    b2r = np.ascontiguousarray(np.asarray(b2, dtype=np.float32).reshape(H // 128, 128).T)
    b3r = np.asarray(b3, dtype=np.float32).reshape(1, D).astype(ml_dtypes.bfloat16)
    mask_np = np.asarray(mask, dtype=np.float32)
    mask3 = mask_np.reshape(CORES, 128, MF)

    in_maps = []
    for c in range(CORES):
        in_maps.append({
            "x": x_bf[c * RPC:(c + 1) * RPC],
            "w1t": w1t, "w2t": w2t, "w3t": w3t,
            "b1r": b1r, "b2r": b2r, "b3r": b3r,
            "maskd": mask3[c],
        })

    LAST_IN_MAPS = in_maps
    res = bass_utils.run_bass_kernel_spmd(nc, in_maps, core_ids=list(range(CORES)))
    LAST_RESULTS = res

    out = np.concatenate([res.results[c]["outd"] for c in range(CORES)], axis=0)
    hi = np.concatenate([res.results[c]["hid"].reshape(-1) for c in range(CORES)])
    hi_mask = hi.view(np.bool_).reshape(512, 512, 512)
    return out, hi_mask
